# revision 40
# baseline (speedup 1.0000x reference)
"""DialogueGCN forward as a Bass/Tile kernel on 8 TRN2 NeuronCores.

Sharding: data-parallel over dialogues (batch). Each core owns 32 contiguous
dialogues; edges never cross dialogues so all graph aggregation is local.

Per-dialogue math (u = source utterance, t = target utterance, band |u-t|<=10):
  scaleT[u, t] = (W_att^T @ x_b^T)[u, t]
  P = exp(scaleT)                  (softmax Z cancels in the masked renorm)
  Shat_{s,dd}[u,t] = P[u,t] * m_s[u] * dir_dd[u,t]   (band cols, 4 variants
                                                      stacked in one tile)
  sums[u] = sum_{s,dd,t} Shat     (stt accum)
  xr[u,:] = x[u,:] / sums[u]
  G[d, (s,dd), t] = sum_u xr[u,d] * Shat_{s,dd}[u,t]  (banded overlap-tiled
                     contract; one N=400 quad matmul per (t-tile, d-chunk))
  h1_st[tau] = sum_{s,dd} w8[s*4+tau*2+dd]^T @ G[:, (s,dd), :]
  h1 = select_by_target_speaker(h1_st0, h1_st1) + root^T @ x^T + bias_r
  qT[u,h2] = (h1^T W2)[u,h2];  h2 = W1^T h1 + qT^T-contract win + b_gc
  hid = relu(Wlin_d^T x^T + Wlin_h^T h2 + b_lin)
  logits = Wfc^T hid + b_fc;  out = log_softmax over 6 classes

Scheduling: three stages per dialogue (A: attention scores, B: banded
aggregation, C: projections+head) issued with a software-pipeline skew
C(b-2), B(b-1), A(b) so every engine queue always holds ready work.
DMA discipline: every HWDGE DMA serializes ~625ns on the shared HWDGE
device, so loads are batched (masks once per core, x in dialogue groups,
one output store; host reshuffles the final layout).
"""

import os

import numpy as np
import ml_dtypes

import concourse.bass as bass
import concourse.mybir as mybir
import concourse.tile as tile
from concourse import bass_utils

SEQ, BATCH, D, H, NCLS = 300, 256, 200, 128, 6
WP = WF = 10
NCORES = 8
BPC = BATCH // NCORES   # dialogues per core
GRP = 4                 # dialogues per x-load group
NGRP = BPC // GRP
UT = [(0, 128), (128, 128), (256, 44)]     # aligned seq tiles (qt/ph2/transp)
OT = [(0, 110), (90, 120), (190, 110)]     # overlap u-tiles (banded contract)
BT = [(0, 120), (80, 140), (180, 120)]     # band col range per overlap tile
BW = 140                                    # stacked-Shat column stride
TT = [0, 100, 200]                          # G t-tile starts (width 100)
WR = [(0, 138), (118, 148), (246, 54)]     # ph2 band col range per aligned tile
F32 = mybir.dt.float32
F32R = mybir.dt.float32r
BF16 = mybir.dt.bfloat16
NPBF = ml_dtypes.bfloat16

_CACHE = {}


def _split_multiwaits(nc, max_waits=1):
    """walrus in this container rejects >1 sem wait on an instruction
    ("Too many sync wait commands"); hoist extras onto preceding NOPs."""
    n = 0
    for f in nc.m.functions:
        for b in f.blocks:
            newlist = []
            changed = False
            for ins in b.instructions:
                si = ins.sync_info
                if si is not None and si.on_wait is not None and len(si.on_wait) > max_waits:
                    waits = list(si.on_wait)
                    for w in waits[max_waits:]:
                        n += 1
                        nop = mybir.InstNoOp(name=f"waitsplit-{n}", ins=[], outs=[])
                        nop.engine = ins.engine
                        nop.sync_info = mybir.SyncInfo(on_wait=[w], on_update=[])
                        newlist.append(nop)
                        nc.inst_map[nop.name] = nop
                    ins.sync_info = mybir.SyncInfo(
                        on_wait=waits[:max_waits],
                        on_update=list(si.on_update) if si.on_update else [],
                    )
                    changed = True
                newlist.append(ins)
            if changed:
                b.instructions = newlist
    return n


def _build_program():
    nc = bass.Bass("TRN2", num_devices=NCORES)

    ap = {}
    def din(name, shape, dt=F32):
        ap[name] = nc.dram_tensor(name, shape, dt, kind="ExternalInput").ap()

    din("xt", (BPC, D, SEQ), F32R)     # per-dialogue x^T (d-major)
    din("xnb", (BPC, SEQ, D), BF16)    # per-dialogue x (seq-major) bf16
    din("mskt", (2, SEQ, BPC))         # speaker one-hot masks, seq-major
    din("msk0r", (1, BPC * SEQ))       # speaker-0 mask, one partition row
    for dd in range(2):
        for k in range(3):
            din(f"dirb{dd}{k}", (OT[k][1], BT[k][1]), BF16)
    for k in range(3):
        din(f"winb{k}", (UT[k][1], WR[k][1]), BF16)
    for k in range(3):
        din(f"winbo{k}", (OT[k][1], BT[k][1]), BF16)
    din("watt", (2, 100, SEQ), F32R)
    din("w8b", (2, 100, 8, H), F32R)
    din("rootm", (2, 100, H), F32R)
    din("wlind", (2, 100, H), F32R)
    din("w1m", (H, H), BF16)
    din("w2m", (H, H), BF16)
    din("wlinh", (H, H), F32R)
    din("wfc", (H, NCLS), F32R)
    din("ident", (128, 128))
    din("brc", (H, 1))
    din("bgc", (H, 1))
    din("blc", (H, 1))
    din("bfc", (NCLS, 1))
    out = nc.dram_tensor("out", (128, BPC * 3 * NCLS), F32,
                         kind="ExternalOutput").ap()

    repeat = int(os.environ.get("BASS_REPEAT", "1"))
    from contextlib import ExitStack
    with tile.TileContext(nc) as tc:
        with ExitStack() as ctx:
            pools = _mk_pools(tc, ctx)
            if repeat > 1:
                with tc.For_i(0, repeat, 1):
                    _body(nc, tc, ap, out, pools)
            else:
                _body(nc, tc, ap, out, pools)

    _split_multiwaits(nc)
    return nc


def _mk_pools(tc, ctx):
    return dict(
        cpool=ctx.enter_context(tc.tile_pool(name="const", bufs=1)),
        io=ctx.enter_context(tc.tile_pool(name="io", bufs=3)),
        wk=ctx.enter_context(tc.tile_pool(name="wk", bufs=4)),
        ps_big=ctx.enter_context(tc.tile_pool(name="ps_big", bufs=4, space="PSUM")),
        ps_ph1=ctx.enter_context(tc.tile_pool(name="ps_ph1", bufs=2, space="PSUM")),
        ps_g=ctx.enter_context(tc.tile_pool(name="ps_g", bufs=2, space="PSUM")),
    )


class _Consts:
    pass


def _load_consts(nc, cpool, ap):
    """Only what dialogue 0's scale/proot need goes on HWDGE up front —
    everything else defers to _load_consts_late after the group-0 loads."""
    c = _Consts()
    c.watt = []
    for ch in range(2):
        t = cpool.tile([100, SEQ], F32R, name=f"c_watt_{ch}")
        nc.sync.dma_start(t[:], ap["watt"][ch])
        c.watt.append(t)
    c.root = cpool.tile([100, 2 * H], F32R, name="c_root")
    nc.sync.dma_start(c.root.rearrange("p (c h) -> p c h", c=2),
                      ap["rootm"].transpose([1, 0, 2]))
    c.lout = cpool.tile([128, BPC * 3 * NCLS], F32, name="c_lout")
    return c


def _load_consts_late(nc, cpool, ap, c):
    """Bulk constants: mskt on HWDGE (needed within ~6us), the rest on the
    software DGE (Pool queue) so they overlap the HWDGE input loads during
    pipeline fill."""
    dma = nc.gpsimd.dma_start
    c.brc = cpool.tile([H, 1], F32, name="c_brc")
    dma(c.brc[:], ap["brc"][:])
    c.bgc = cpool.tile([H, 1], F32, name="c_bgc")
    dma(c.bgc[:], ap["bgc"][:])
    c.blc = cpool.tile([H, 1], F32, name="c_blc")
    dma(c.blc[:], ap["blc"][:])
    c.bfc = cpool.tile([NCLS, 1], F32, name="c_bfc")
    dma(c.bfc[:], ap["bfc"][:])
    c.mskt = {}
    for s in range(2):
        for k in range(3):
            o0, orows = OT[k]
            t = cpool.tile([128, BPC], F32, name=f"c_mskt{s}{k}")
            nc.sync.dma_start(t[:orows, :], ap["mskt"][s, o0:o0 + orows, :])
            c.mskt[(s, k)] = t
    c.dirb = {}
    for dd in range(2):
        for k in range(3):
            orows, bc = OT[k][1], BT[k][1]
            t = cpool.tile([128, bc], BF16, name=f"c_dirb{dd}{k}")
            dma(t[:orows, :], ap[f"dirb{dd}{k}"][:, :])
            c.dirb[(dd, k)] = t
    c.winbo = []
    for k in range(3):
        orows, bc = OT[k][1], BT[k][1]
        t = cpool.tile([128, bc], BF16, name=f"c_winbo{k}")
        dma(t[:orows, :], ap[f"winbo{k}"][:, :])
        c.winbo.append(t)
    c.w8 = []
    for ch in range(2):
        t = cpool.tile([100, 8 * H], F32R, name=f"c_w8_{ch}")
        dma(t.rearrange("p (r h) -> p r h", r=8), ap["w8b"][ch])
        c.w8.append(t)
    c.winb = []
    for k in range(3):
        uk, wkk = UT[k][1], WR[k][1]
        t = cpool.tile([128, wkk], BF16, name=f"c_winb{k}")
        dma(t[:uk, :], ap[f"winb{k}"][:, :])
        c.winb.append(t)
    c.wlind = cpool.tile([100, 2 * H], F32R, name="c_wlind")
    dma(c.wlind.rearrange("p (c h) -> p c h", c=2),
        ap["wlind"].transpose([1, 0, 2]))
    c.w1 = cpool.tile([H, H], BF16, name="c_w1")
    dma(c.w1[:], ap["w1m"][:])
    c.w2 = cpool.tile([H, H], BF16, name="c_w2")
    dma(c.w2[:], ap["w2m"][:])
    c.wlinh = cpool.tile([H, H], F32R, name="c_wlinh")
    dma(c.wlinh[:], ap["wlinh"][:])
    c.wfc = cpool.tile([H, NCLS], F32R, name="c_wfc")
    dma(c.wfc[:], ap["wfc"][:])
    c.ident = cpool.tile([128, 128], F32, name="c_ident")
    dma(c.ident[:], ap["ident"][:])


def _body(nc, tc, ap, out, pools):
    cpool = pools["cpool"]
    io = pools["io"]
    wk = pools["wk"]
    ps_big = pools["ps_big"]
    ps_ph1 = pools["ps_ph1"]
    ps_g = pools["ps_g"]

    AF = mybir.ActivationFunctionType
    OP = mybir.AluOpType
    MM = nc.tensor.matmul

    c = _load_consts(nc, cpool, ap)
    st = {}          # per-dialogue state: b -> dict of tiles
    grp = {}         # group tiles: g -> (xtg, xng, tmbg)

    def stage_a1(b):
        """Loads, scale matmuls + exp, proot. PE work fills the select-chain
        bubble of C(b-2)."""
        g, di = divmod(b, GRP)
        if di == 0:
            b0g = g * GRP
            xtg = []
            for ch in range(2):
                t = io.tile([100, GRP * SEQ], F32R, name=f"xtg{ch}")
                nc.sync.dma_start(
                    t.rearrange("p (b s) -> p b s", b=GRP),
                    ap["xt"][b0g:b0g + GRP, ch * 100:(ch + 1) * 100, :]
                    .transpose([1, 0, 2]))
                xtg.append(t)
            xng = []
            for k in range(3):
                o0, orows = OT[k]
                t = io.tile([128, GRP * D], BF16, name=f"xng{k}")
                nc.sync.dma_start(
                    t[:orows, :].rearrange("p (b d) -> p b d", b=GRP),
                    ap["xnb"][b0g:b0g + GRP, o0:o0 + orows, :]
                    .transpose([1, 0, 2]))
                xng.append(t)
            # target-speaker mask rows broadcast to all partitions, whole group
            tmbg = io.tile([128, GRP * SEQ], F32, name="tmbg")
            nc.sync.dma_start(
                tmbg[:, :],
                ap["msk0r"][0, b0g * SEQ:(b0g + GRP) * SEQ].unsqueeze(0)
                .partition_broadcast(128))
            grp.pop(g - 2, None)
            grp[g] = (xtg, xng, tmbg)
            if b == 0:
                _load_consts_late(nc, cpool, ap, c)
        xtg, xng, tmbg = grp[g]
        d = st[b] = {}
        d["xt"] = [xtg[ch][:, di * SEQ:(di + 1) * SEQ] for ch in range(2)]
        d["tmb"] = tmbg.bitcast(mybir.dt.int32)[:, di * SEQ:(di + 1) * SEQ]
        d["xng"] = [xng[k][:OT[k][1], di * D:(di + 1) * D] for k in range(3)]

        # scale + exp (band cols only) per overlap tile
        d["p"] = []
        for k in range(3):
            o0, orows = OT[k]
            tb0, bc = BT[k]
            psc = ps_big.tile([128, SEQ], F32, name="pbig", tag="pbig")
            for ch in range(2):
                MM(psc[:orows, :], c.watt[ch][:, o0:o0 + orows], d["xt"][ch],
                   start=(ch == 0), stop=(ch == 1))
            p = wk.tile([128, BW], BF16, name=f"p{k}")
            nc.scalar.activation(p[:orows, :bc], psc[:orows, tb0:tb0 + bc],
                                 AF.Exp)
            d["p"].append(p)

        # x @ root (held in SBUF until stage C's h1f)
        proot = ps_big.tile([128, SEQ], F32, name="pbig", tag="pbig")
        for ch in range(2):
            MM(proot[:H, :], c.root[:, ch * H:(ch + 1) * H], d["xt"][ch],
               start=(ch == 0), stop=(ch == 1))
        d["proot"] = wk.tile([H, SEQ], F32, name="proot")
        nc.scalar.activation(d["proot"][:], proot[:H, :], AF.Identity,
                             bias=c.brc[:])

    def stage_a2(b):
        """Shat quad-stacks + row sums + xr — DVE/Pool only, no PE work."""
        d = st[b]
        d["ss"] = []
        d["xr"] = []
        for k in range(3):
            o0, orows = OT[k]
            tb0, bc = BT[k]
            ss = wk.tile([128, 4 * BW], BF16, name=f"ss{k}")
            pm = []
            for s in range(2):
                t = wk.tile([128, BW], BF16, name=f"pm{s}{k}")
                nc.vector.tensor_scalar_mul(t[:orows, :bc],
                                            d["p"][k][:orows, :bc],
                                            c.mskt[(s, k)][:orows, b:b + 1])
                pm.append(t)
            for s in range(2):
                for dd in range(2):
                    v = s * 2 + dd
                    nc.gpsimd.tensor_tensor(
                        ss[:orows, v * BW:v * BW + bc], pm[s][:orows, :bc],
                        c.dirb[(dd, k)][:orows, :], op=OP.mult)
            d["ss"].append(ss)
            # row sums of P*win (out is a throwaway band tile)
            scr = wk.tile([128, BW], BF16, name=f"scr{k}")
            sm = wk.tile([128, 1], F32, name=f"sm{k}")
            nc.vector.scalar_tensor_tensor(
                scr[:orows, :bc], d["p"][k][:orows, :bc], 1.0,
                c.winbo[k][:orows, :],
                op0=OP.mult, op1=OP.mult, accum_out=sm[:orows, :])
            rc = wk.tile([128, 1], F32, name=f"rc{k}")
            nc.vector.reciprocal(rc[:orows, :], sm[:orows, :])
            xr = wk.tile([128, D], BF16, name=f"xr{k}")
            nc.vector.tensor_scalar_mul(xr[:orows, :], d["xng"][k],
                                        rc[:orows, :])
            d["xr"].append(xr)

    def stage_b(b):
        """G quad: one N=400 matmul per (t-tile, d-chunk); copy to f32r SBUF.
        Fills the h2-activation bubble of C(b-1)."""
        d = st[b]
        d["gb"] = []
        if b % 2 == 0:
            copy_eng = [nc.scalar.copy, nc.vector.tensor_copy,
                        nc.scalar.copy, nc.scalar.copy,
                        nc.vector.tensor_copy, nc.scalar.copy]
        else:
            copy_eng = [nc.scalar.copy, nc.vector.tensor_copy,
                        nc.vector.tensor_copy, nc.scalar.copy,
                        nc.vector.tensor_copy, nc.scalar.copy]
        gi = 0
        for ch in range(2):
            gbig = wk.tile([128, 3 * 4 * 100], F32R, name=f"gb{ch}")
            for k in range(3):
                o0, orows = OT[k]
                lo = TT[k] - BT[k][0]
                pg = ps_g.tile([128, 400], F32, name="psg", tag="psg")
                MM(pg[:100, :],
                   d["xr"][k][:orows, ch * 100:(ch + 1) * 100],
                   d["ss"][k].rearrange("p (v c) -> p v c", c=BW)
                   [:orows, :, lo:lo + 100],
                   start=True, stop=True)
                copy_eng[gi](gbig[:100, k * 400:(k + 1) * 400], pg[:100, :])
                gi += 1
            d["gb"].append(gbig)

    def stage_c1(b):
        """ph1 (2µs of ready PE work opens the iteration) + speaker select."""
        d = st[b]
        ph1 = []
        for tau in range(2):
            pt0 = ps_ph1.tile([128, 3 * H], F32, name="ph1", tag="ph1")
            pt = pt0[:H, :SEQ]
            first = True
            for s in range(2):
                for dd in range(2):
                    r = s * 4 + tau * 2 + dd
                    v = s * 2 + dd
                    for ch in range(2):
                        MM(pt[:, :], c.w8[ch][:, r * H:(r + 1) * H],
                           d["gb"][ch].rearrange("p (k v c) -> p k v c",
                                                 k=3, v=4)[:100, :, v, :],
                           start=first, stop=(s == 1 and dd == 1 and ch == 1))
                        first = False
            ph1.append(pt)

        d["ph1"] = ph1

    def stage_c1b(b):
        d = st[b]
        ph1 = d.pop("ph1")
        nc.vector.copy_predicated(ph1[1][:], d["tmb"], ph1[0][:])
        d["h1f"] = wk.tile([H, SEQ], BF16, name="h1f")
        nc.vector.tensor_tensor(d["h1f"][:], ph1[1][:], d["proot"][:],
                                op=OP.add)

    def stage_c2(b):
        """qT = h1^T W2 (bf16), then h2 = W1^T h1 + banded-qT + b_gc."""
        d = st[b]
        sb_h1f = d["h1f"]
        pq = ps_ph1.tile([128, 3 * H], F32, name="ph1", tag="ph1")
        for k in range(3):
            u0, uk = UT[k]
            MM(pq[:uk, k * H:(k + 1) * H], sb_h1f[:, u0:u0 + uk], c.w2[:],
               start=True, stop=True)
        qt = wk.tile([128, 3 * H], BF16, name="qt")
        nc.vector.tensor_copy(qt[:], pq[:])

        ph2 = ps_big.tile([128, SEQ], F32, name="pbig", tag="pbig")
        MM(ph2[:H, :], c.w1[:], sb_h1f[:], start=True, stop=False)
        for k in range(3):
            u0, uk = UT[k]
            w0, wkk = WR[k]
            MM(ph2[:H, w0:w0 + wkk], qt[:uk, k * H:(k + 1) * H],
               c.winb[k][:uk, :], start=False, stop=(k == 2))
        d["h2"] = wk.tile([H, SEQ], F32R, name="h2")
        nc.scalar.activation(d["h2"][:], ph2[:H, :], AF.Identity, bias=c.bgc[:])

    def stage_c3(b):
        """hid = relu(Wlin_d x + Wlin_h h2 + b); logits."""
        d = st[b]
        phid = ps_big.tile([128, SEQ], F32, name="pbig", tag="pbig")
        for ch in range(2):
            MM(phid[:H, :], c.wlind[:, ch * H:(ch + 1) * H], d["xt"][ch],
               start=(ch == 0), stop=False)
        MM(phid[:H, :], c.wlinh[:], d["h2"][:], start=False, stop=True)
        sb_hid = wk.tile([H, SEQ], F32R, name="hid")
        nc.scalar.activation(sb_hid[:], phid[:H, :], AF.Relu, bias=c.blc[:])

        plg = ps_big.tile([128, SEQ], F32, name="pbig", tag="pbig")
        MM(plg[:NCLS, :], c.wfc[:], sb_hid[:], start=True, stop=True)
        d["lg"] = wk.tile([NCLS, SEQ], F32, name="lg")
        nc.scalar.activation(d["lg"][:], plg[:NCLS, :], AF.Identity,
                             bias=c.bfc[:])

    def stage_c4(b):
        """Transpose logits into the batched l_out staging tile."""
        d = st.pop(b)
        plt = ps_big.tile([128, SEQ], F32, name="pbig", tag="pbig")
        for k in range(3):
            u0, uk = UT[k]
            nc.tensor.transpose(plt[:uk, k * NCLS:(k + 1) * NCLS],
                                d["lg"][:, u0:u0 + uk], c.ident[:NCLS, :NCLS])
        nc.vector.tensor_copy(c.lout[:, b * 18:b * 18 + 12], plt[:, 0:12])
        nc.vector.tensor_copy(c.lout[:44, b * 18 + 12:b * 18 + 18],
                              plt[:44, 12:18])

    # ---- stage 2: batched log-softmax over classes, two halves so the
    # first half overlaps the tail of the dialogue pipeline ----
    GG = BPC * 3  # 96 groups of 6 classes
    osb = cpool.tile([128, GG * NCLS], F32, name="c_osb")
    s2 = {}
    for h in range(2):
        s2[h] = dict(
            m=cpool.tile([128, GG // 2], F32, name=f"c_m96_{h}"),
            e=cpool.tile([128, GG * NCLS // 2], F32, name=f"c_esb_{h}"),
            x=cpool.tile([128, GG * NCLS // 2], F32, name=f"c_e2sb_{h}"),
            s=cpool.tile([128, GG // 2], F32, name=f"c_s96_{h}"),
            z=cpool.tile([128, GG // 2], F32, name=f"c_lnz_{h}"),
            f=cpool.tile([128, GG // 2], F32, name=f"c_lsm_{h}"),
        )

    def stage2_half(h):
        ng = GG // 2
        l3 = c.lout[:, h * ng * NCLS:(h + 1) * ng * NCLS].rearrange(
            "p (g c) -> p g c", c=NCLS)
        t = s2[h]
        nc.vector.reduce_max(t["m"][:], l3, axis=mybir.AxisListType.X)
        e3 = t["e"].rearrange("p (g c) -> p g c", c=NCLS)
        for cc in range(NCLS):
            eng = nc.vector if cc % 2 == 0 else nc.gpsimd
            eng.tensor_tensor(e3[:, :, cc], l3[:, :, cc], t["m"][:],
                              op=OP.subtract)
        nc.scalar.activation(t["x"][:], t["e"][:], AF.Exp)
        nc.vector.reduce_sum(t["s"][:],
                             t["x"].rearrange("p (g c) -> p g c", c=NCLS),
                             axis=mybir.AxisListType.X)
        nc.scalar.activation(t["z"][:], t["s"][:], AF.Ln)
        nc.vector.tensor_tensor(t["f"][:], t["m"][:], t["z"][:], op=OP.add)
        o3 = osb[:, h * ng * NCLS:(h + 1) * ng * NCLS].rearrange(
            "p (g c) -> p g c", c=NCLS)
        for cc in range(NCLS):
            eng = nc.vector if cc % 2 == 0 else nc.gpsimd
            eng.tensor_tensor(o3[:, :, cc], l3[:, :, cc], t["f"][:],
                              op=OP.subtract)

    # Software-pipelined issue. Skews: a(i), b(i-1), c1/c2(i-2), c3/c4(i-3).
    # Within an iteration, stages are interleaved so each engine's in-order
    # queue sees instructions sorted by dependency-readiness time.
    for i in range(BPC + 6):
        c12 = 5 <= i <= BPC + 4
        c34 = 6 <= i
        if c34:
            stage_c3(i - 6)
        if c12:
            stage_c1(i - 5)
        if i < BPC:
            stage_a1(i)
        if c34:
            stage_c4(i - 6)
        if i < BPC:
            stage_a2(i)
        if c12:
            stage_c1b(i - 5)
        if 3 <= i <= BPC + 2:
            stage_b(i - 3)
        if c12:
            stage_c2(i - 5)
        if i == BPC // 2 + 6:
            stage2_half(0)
            nc.sync.dma_start(out[:, :BPC * 3 * NCLS // 2],
                              osb[:, :BPC * 3 * NCLS // 2])
    stage2_half(1)
    nc.sync.dma_start(out[:, BPC * 3 * NCLS // 2:],
                      osb[:, BPC * 3 * NCLS // 2:])



def _host_prep(inputs):
    feats = np.asarray(inputs["features"], dtype=np.float32)    # (300,256,200)
    spk = np.asarray(inputs["speakers"])                        # (300,256)
    W_att = np.asarray(inputs["W_att"], dtype=np.float32)
    basis = np.asarray(inputs["basis"], dtype=np.float32)
    comp = np.asarray(inputs["comp"], dtype=np.float32)
    root = np.asarray(inputs["root"], dtype=np.float32)
    bias_r = np.asarray(inputs["bias_r"], dtype=np.float32)
    W1 = np.asarray(inputs["W1"], dtype=np.float32)
    W2 = np.asarray(inputs["W2"], dtype=np.float32)
    b_gc = np.asarray(inputs["b_gc"], dtype=np.float32)
    W_lin = np.asarray(inputs["W_lin"], dtype=np.float32)
    b_lin = np.asarray(inputs["b_lin"], dtype=np.float32)
    W_fc = np.asarray(inputs["W_fc"], dtype=np.float32)
    b_fc = np.asarray(inputs["b_fc"], dtype=np.float32)

    i = np.arange(SEQ)[:, None]
    j = np.arange(SEQ)[None, :]
    win = (j >= i - WP) & (j <= i + WF)
    dir0 = (win & (i < j)).astype(np.float32)
    dir1 = (win & (i >= j)).astype(np.float32)
    winm = win.astype(np.float32)

    shared = {}
    for dd, dm in ((0, dir0), (1, dir1)):
        for k in range(3):
            o0, orows = OT[k]
            tb0, bc = BT[k]
            shared[f"dirb{dd}{k}"] = np.ascontiguousarray(
                dm[o0:o0 + orows, tb0:tb0 + bc]).astype(NPBF)
    for k in range(3):
        u0, uk = UT[k]
        w0, wkk = WR[k]
        shared[f"winb{k}"] = np.ascontiguousarray(
            winm[u0:u0 + uk, w0:w0 + wkk]).astype(NPBF)
    for k in range(3):
        o0, orows = OT[k]
        tb0, bc = BT[k]
        shared[f"winbo{k}"] = np.ascontiguousarray(
            winm[o0:o0 + orows, tb0:tb0 + bc]).astype(NPBF)

    w8 = np.einsum("rb,bdh->rdh", comp, basis).astype(np.float32)
    shared["w8b"] = np.ascontiguousarray(
        w8.reshape(8, 2, 100, H).transpose(1, 2, 0, 3))
    shared["watt"] = np.ascontiguousarray(W_att.reshape(2, 100, SEQ))
    shared["rootm"] = np.ascontiguousarray(root.reshape(2, 100, H))
    shared["wlind"] = np.ascontiguousarray(W_lin[:D]).reshape(2, 100, H)
    shared["w1m"] = W1.astype(NPBF)
    shared["w2m"] = W2.astype(NPBF)
    shared["wlinh"] = np.ascontiguousarray(W_lin[D:])
    shared["wfc"] = W_fc
    shared["ident"] = np.eye(128, dtype=np.float32)
    shared["brc"] = bias_r.reshape(H, 1)
    shared["bgc"] = b_gc.reshape(H, 1)
    shared["blc"] = b_lin.reshape(H, 1)
    shared["bfc"] = b_fc.reshape(NCLS, 1)

    in_maps = []
    for cc in range(NCORES):
        bs = slice(cc * BPC, (cc + 1) * BPC)
        fb = feats[:, bs, :]                                    # (300,32,200)
        xt = np.ascontiguousarray(fb.transpose(1, 2, 0))        # (32,200,300)
        xnb = np.ascontiguousarray(fb.transpose(1, 0, 2)).astype(NPBF)
        sp = spk[:, bs].T                                       # (32,300)
        mskt = np.ascontiguousarray(
            np.stack([(sp == 0).T, (sp == 1).T]).astype(np.float32))
        msk0r = np.ascontiguousarray(
            (sp == 0).astype(np.float32).reshape(1, BPC * SEQ))
        m = {"xt": xt, "xnb": xnb, "mskt": mskt, "msk0r": msk0r}
        m.update(shared)
        in_maps.append(m)
    return in_maps


def get_program():
    if "nc" not in _CACHE:
        _CACHE["nc"] = _build_program()
    return _CACHE["nc"]


def kernel(**inputs):
    nc = get_program()
    in_maps = _host_prep(inputs)
    res = bass_utils.run_bass_kernel_spmd(nc, in_maps, core_ids=list(range(NCORES)))
    outs = []
    for cc in range(NCORES):
        a = res.results[cc]["out"].reshape(128, BPC, 3, NCLS)
        full = np.concatenate(
            [a[:UT[0][1], :, 0, :], a[:UT[1][1], :, 1, :], a[:UT[2][1], :, 2, :]],
            axis=0)                                             # (300, 32, 6)
        outs.append(full.transpose(1, 0, 2).reshape(BPC * SEQ, NCLS))
    return np.concatenate(outs, axis=0)
